# revision 1
# baseline (speedup 1.0000x reference)
"""BPLoss Trainium2 kernel: 8-core SPMD over the detection (N) axis.

v3 design (best HW-verified: 227.6 us/core, rel err 1.6e-7).

Per core (shard of R=12544 rows; partition p owns rows p*98..p*98+97, so
each group DMA reads one contiguous 28 KiB run per partition):
  - 14 groups x [128, 7, 1024] f32 plain HWDGE DMAs (3.5 MiB each) on
    the sync queue stream class_scores at line rate
  - masking, per 128-row tile, in place:
      DVE tiles:  masked = (iota != label) * cs   (one fused
                  scalar_tensor_tensor; label is a per-partition scalar)
      Pool tiles: ScalarE builds v = Relu(BIG - BIG*Square(iota-label))
                  (= BIG one-hot at the label column) in two activation
                  passes, GpSimd subtracts it from cs
  - one reduce_max per group produces 7 row-max columns at once
  - epilogue: Ln on ScalarE, fused multiply-accumulate dot products for
    sum((z+r)*log_max) and sum(z*||xywh - gt_xywh[idx]||^2)
Host: gathers the tiny gt tables per row (labels, gt_xywh[idx]), shards,
pads core 7, sums the 8x[128,2] partials, combines -A + exp(-B).
"""
import numpy as np
import concourse.bass as bass
import concourse.tile as tile
from concourse import bacc, mybir
from concourse.bass_utils import run_bass_kernel_spmd

N, C, M = 100000, 1024, 128
NCORES = 8
T = 98              # 128-row tiles per core
R = T * 128         # 12544 rows per core
G = 7               # tiles per DMA group
NG = T // G         # 14 groups
DVE_APPLY = 2       # leading tiles of each group masked on DVE; rest GpSimd
CS_BUFS = 3
V_BUFS = 8
BIG = 1024.0

f32 = mybir.dt.float32
bf16 = mybir.dt.bfloat16
OP = mybir.AluOpType
AF = mybir.ActivationFunctionType
AX = mybir.AxisListType

# packed f32 per-row tables: [label | -label | z | r | xywh | g | iota | BIG]
PF_LAB = 0
PF_NLAB = T
PF_Z = 2 * T
PF_R = 3 * T
PF_XYWH = 4 * T
PF_G = 8 * T
PF_IOTA = 12 * T
PF_BIGC = 12 * T + C
PF_COLS = 12 * T + C + 1


def build_nc(reps=1):
    nc = bacc.Bacc("TRN2", target_bir_lowering=False, debug=False,
                   num_devices=NCORES)
    cs_d = nc.dram_tensor("cs", [128, T * C], f32, kind="ExternalInput").ap()
    pf_d = nc.dram_tensor("pf", [128, PF_COLS], f32, kind="ExternalInput").ap()
    out_d = nc.dram_tensor("out", [128, 2], f32, kind="ExternalOutput").ap()

    with tile.TileContext(nc) as tc:
        with (
            tc.tile_pool(name="const", bufs=1) as constp,
            tc.tile_pool(name="csp", bufs=CS_BUFS) as csp,
            tc.tile_pool(name="vp", bufs=V_BUFS) as vp,
        ):
            pf = constp.tile([128, PF_COLS], f32)
            nc.sync.dma_start(out=pf[:], in_=pf_d[:])
            lab = pf[:, PF_LAB : PF_LAB + T]
            nlab = pf[:, PF_NLAB : PF_NLAB + T]
            z_sb = pf[:, PF_Z : PF_Z + T]
            r_sb = pf[:, PF_R : PF_R + T]
            xywh_sb = pf[:, PF_XYWH : PF_XYWH + 4 * T].rearrange(
                "p (t c) -> p t c", c=4
            )
            g_sb = pf[:, PF_G : PF_G + 4 * T].rearrange("p (t c) -> p t c", c=4)
            iota = pf[:, PF_IOTA : PF_IOTA + C]
            bigc = pf[:, PF_BIGC : PF_BIGC + 1]

            w_sb = constp.tile([128, T], f32)
            nc.vector.tensor_add(w_sb[:], z_sb, r_sb)
            rowmax = constp.tile([128, T], f32)
            lm = constp.tile([128, T], f32)
            out_sb = constp.tile([128, 2], f32)
            scr = constp.tile([128, T], f32)
            scr2 = constp.tile([128, T], f32)
            diff = constp.tile([128, T, 4], f32)
            dsum = constp.tile([128, T], f32)

            for rep in range(reps):
                for g in range(NG):
                    t0 = g * G
                    csw = csp.tile([128, G, C], f32)
                    nc.sync.dma_start(
                        out=csw[:],
                        in_=cs_d[:, t0 * C : (t0 + G) * C].rearrange(
                            "p (a c) -> p a c", c=C
                        ),
                    )
                    for h in range(G):
                        t = t0 + h
                        if h < DVE_APPLY:
                            nc.vector.scalar_tensor_tensor(
                                out=csw[:, h, :], in0=iota,
                                scalar=lab[:, t : t + 1],
                                in1=csw[:, h, :],
                                op0=OP.not_equal, op1=OP.mult,
                            )
                        else:
                            u = vp.tile([128, C], f32)
                            nc.scalar.activation(
                                out=u[:], in_=iota, func=AF.Square,
                                scale=1.0, bias=nlab[:, t : t + 1],
                            )
                            nv = vp.tile([128, C], bf16)
                            nc.scalar.activation(
                                out=nv[:], in_=u[:], func=AF.Sign
                            )
                            nc.gpsimd.tensor_tensor(
                                out=csw[:, h, :], in0=csw[:, h, :],
                                in1=nv[:], op=OP.mult,
                            )
                    nc.vector.reduce_max(
                        rowmax[:, t0 : t0 + G], csw[:], axis=AX.X
                    )

                # epilogue: partial sums
                nc.scalar.activation(out=lm[:], in_=rowmax[:], func=AF.Ln)
                nc.vector.scalar_tensor_tensor(
                    out=scr[:], in0=w_sb[:], scalar=0.0, in1=lm[:],
                    op0=OP.bypass, op1=OP.mult, accum_out=out_sb[:, 0:1],
                )
                nc.vector.tensor_sub(diff[:], xywh_sb, g_sb)
                nc.vector.tensor_mul(diff[:], diff[:], diff[:])
                nc.vector.reduce_sum(dsum[:], diff[:], axis=AX.X)
                nc.vector.scalar_tensor_tensor(
                    out=scr2[:], in0=z_sb, scalar=0.0, in1=dsum[:],
                    op0=OP.bypass, op1=OP.mult, accum_out=out_sb[:, 1:2],
                )
            nc.sync.dma_start(out=out_d[:], in_=out_sb[:])

    nc.compile()
    return nc


def make_in_maps(class_scores, xywh, z, r, nearest_gt_idx, gt_class_labels, gt_xywh):
    cs = np.ascontiguousarray(np.asarray(class_scores, dtype=np.float32))
    xywh = np.ascontiguousarray(np.asarray(xywh, dtype=np.float32))
    z = np.ascontiguousarray(np.asarray(z, dtype=np.float32))
    r = np.ascontiguousarray(np.asarray(r, dtype=np.float32))
    idx = np.asarray(nearest_gt_idx).astype(np.int64)
    labels = np.asarray(gt_class_labels).astype(np.float32)[idx]       # [N]
    gx = np.asarray(gt_xywh, dtype=np.float32)[idx]                    # [N,4]

    iota_row = np.arange(C, dtype=np.float32)[None, :]
    in_maps = []
    for c in range(NCORES):
        lo, hi = c * R, (c + 1) * R
        if hi <= N:
            cs_s = cs[lo:hi]
            lab_s, z_s, r_s = labels[lo:hi], z[lo:hi], r[lo:hi]
            xywh_s, gx_s = xywh[lo:hi], gx[lo:hi]
        else:
            n_real = N - lo
            cs_s = np.ones((R, C), dtype=np.float32)
            cs_s[:n_real] = cs[lo:]
            lab_s = np.zeros(R, np.float32); lab_s[:n_real] = labels[lo:]
            z_s = np.zeros(R, np.float32); z_s[:n_real] = z[lo:]
            r_s = np.zeros(R, np.float32); r_s[:n_real] = r[lo:]
            xywh_s = np.zeros((R, 4), np.float32); xywh_s[:n_real] = xywh[lo:]
            gx_s = np.zeros((R, 4), np.float32); gx_s[:n_real] = gx[lo:]
        pf = np.empty((128, PF_COLS), dtype=np.float32)
        pf[:, PF_LAB : PF_LAB + T] = lab_s.reshape(128, T)
        pf[:, PF_NLAB : PF_NLAB + T] = -lab_s.reshape(128, T)
        pf[:, PF_Z : PF_Z + T] = z_s.reshape(128, T)
        pf[:, PF_R : PF_R + T] = r_s.reshape(128, T)
        pf[:, PF_XYWH : PF_XYWH + 4 * T] = xywh_s.reshape(128, 4 * T)
        pf[:, PF_G : PF_G + 4 * T] = gx_s.reshape(128, 4 * T)
        pf[:, PF_IOTA : PF_IOTA + C] = iota_row
        pf[:, PF_BIGC] = BIG
        in_maps.append({"cs": cs_s.reshape(128, T * C), "pf": pf})
    return in_maps


def combine_outputs(outs):
    """outs: list of [128, 2] per-core partials -> final [1] float32."""
    partA = float(sum(o[:, 0].astype(np.float64).sum() for o in outs))
    partB = float(sum(o[:, 1].astype(np.float64).sum() for o in outs))
    with np.errstate(over="ignore", under="ignore"):
        tps = np.exp(-partB)
    val = -partA + tps
    return np.array([val], dtype=np.float32)


_NC_CACHE = None


def get_nc():
    global _NC_CACHE
    if _NC_CACHE is None:
        _NC_CACHE = build_nc()
    return _NC_CACHE


def kernel(**inputs) -> np.ndarray:
    nc = get_nc()
    in_maps = make_in_maps(**inputs)
    res = run_bass_kernel_spmd(nc, in_maps, core_ids=list(range(NCORES)))
    return combine_outputs([res.results[c]["out"] for c in range(NCORES)])



# revision 10
# speedup vs baseline: 1.8529x; 1.8529x over previous
"""BPLoss Trainium2 kernel: 8-core SPMD over the detection (N) axis.

v4 design: fp16 streaming + maskless block-max + host-gathered label value.

Per core (shard of R=12544 rows; partition p owns rows p*98..p*98+97):
  - class_scores uploaded as fp16 (half the HBM traffic of v3), shifted and
    scaled on host to (cs - 1) * 1024 so the row max (~1 - 1e-3 for uniform
    scores) lands near magnitude ~1 with full fp16 relative precision
    instead of near 1.0 where fp16 spacing (4.9e-4) would swamp log(max).
    The device undoes it inside the Ln: log(masked/1024 + 1).
  - NO mask pass over the [N, 1024] matrix.  Instead, per 128-row tile the
    DVE computes 8 per-row block maxes S[k] = max(cs[:, 128k:128k+128]) in a
    single strided reduce_max per 7-tile group (fp16 2x mode).
  - Masked row-max reconstruction (exact unless the 2nd-largest element
    shares its 128-col block with the label AND the label is the argmax,
    ~25 rows in 100k, error ~1e-3 each):
        M1 = max_k S[k]
        Vd = max_k (S[k] + P8[k])   with P8 = -2 at the label's block
        masked = (v == M1) ? Vd : M1
    where v = cs_fp16[row, label] is gathered on host (O(N)) and P8 is a
    host-built [T, 8] fp16 penalty table.
  - epilogue: Ln on ScalarE, fused multiply-accumulate dot products for
    sum((z+r)*log_masked) and sum(z*||xywh - gt_xywh[idx]||^2).
Host: gathers the tiny gt tables per row, shards, pads core 7, sums the
8x[128,2] partials, combines -A + exp(-B).
"""
import numpy as np
import concourse.bass as bass
import concourse.tile as tile
from concourse import bacc, mybir
from concourse.bass_utils import run_bass_kernel_spmd

N, C, M = 100000, 1024, 128
NCORES = 8
T = 98              # 128-row tiles per core
R = T * 128         # 12544 rows per core
G = 7               # tiles per DMA group
NG = T // G         # 14 groups
NB = 8              # per-row column blocks (classes) of C//NB = 128 cols
CS_BUFS = 4
BULK = "reduce"     # "reduce": strided reduce_max / "tree": tensor_tensor max
SCALE = 1024.0      # host uploads (cs - 1) * SCALE
PEN = -4096.0       # block penalty; dominates any real shifted score (>= -1024)

f16 = mybir.dt.float16
f32 = mybir.dt.float32
OP = mybir.AluOpType
AF = mybir.ActivationFunctionType
AX = mybir.AxisListType

# packed f32 per-row tables: [v | z | r | xywh | g]
PF_V = 0
PF_Z = T
PF_R = 2 * T
PF_XYWH = 3 * T
PF_G = 7 * T
PF_COLS = 11 * T


def build_nc(reps=1):
    nc = bacc.Bacc("TRN2", target_bir_lowering=False, debug=False,
                   num_devices=NCORES)
    cs_d = nc.dram_tensor("cs", [128, T * C], f16, kind="ExternalInput").ap()
    pf_d = nc.dram_tensor("pf", [128, PF_COLS], f32, kind="ExternalInput").ap()
    p8_d = nc.dram_tensor("p8", [128, T * NB], f16, kind="ExternalInput").ap()
    out_d = nc.dram_tensor("out", [128, 2], f32, kind="ExternalOutput").ap()

    with tile.TileContext(nc) as tc:
        with (
            tc.tile_pool(name="const", bufs=1) as constp,
            tc.tile_pool(name="csp", bufs=CS_BUFS) as csp,
            tc.tile_pool(name="vp", bufs=2) as vp,
        ):
            pf = constp.tile([128, PF_COLS], f32)
            nc.scalar.dma_start(out=pf[:], in_=pf_d[:])
            p8 = constp.tile([128, T, NB], f16)
            nc.scalar.dma_start(out=p8[:], in_=p8_d[:])
            v_sb = pf[:, PF_V : PF_V + T]
            z_sb = pf[:, PF_Z : PF_Z + T]
            r_sb = pf[:, PF_R : PF_R + T]
            xywh_sb = pf[:, PF_XYWH : PF_XYWH + 4 * T].rearrange(
                "p (t c) -> p t c", c=4
            )
            g_sb = pf[:, PF_G : PF_G + 4 * T].rearrange("p (t c) -> p t c", c=4)

            S = constp.tile([128, T, NB], f16)
            S2 = constp.tile([128, T, NB], f16)
            m1 = constp.tile([128, T], f32)
            vd = constp.tile([128, T], f32)
            eq = constp.tile([128, T], f32)
            dd = constp.tile([128, T], f32)
            mm = constp.tile([128, T], f32)
            lm = constp.tile([128, T], f32)
            w_sb = constp.tile([128, T], f32)
            scr = constp.tile([128, T], f32)
            scr2 = constp.tile([128, T], f32)
            diff = constp.tile([128, T, 4], f32)
            dsum = constp.tile([128, T], f32)
            out_sb = constp.tile([128, 2], f32)
            warm = constp.tile([128, 1], f32)
            warm2 = constp.tile([128, 1], f32)

            # preload the Ln activation table while DMAs stream
            nc.vector.memset(warm[:], 1.0)
            nc.scalar.activation(out=warm2[:], in_=warm[:], func=AF.Ln)

            for rep in range(reps):
                # shape-loss term first: independent of class_scores
                nc.vector.tensor_add(w_sb[:], z_sb, r_sb)
                nc.vector.tensor_sub(diff[:], xywh_sb, g_sb)
                nc.vector.tensor_mul(diff[:], diff[:], diff[:])
                nc.vector.reduce_sum(dsum[:], diff[:], axis=AX.X)
                nc.vector.scalar_tensor_tensor(
                    out=scr2[:], in0=z_sb, scalar=0.0, in1=dsum[:],
                    op0=OP.bypass, op1=OP.mult, accum_out=out_sb[:, 1:2],
                )

                for g in range(NG):
                    t0 = g * G
                    if BULK == "reduce":
                        csw = csp.tile([128, G, NB, C // NB], f16)
                        nc.sync.dma_start(
                            out=csw[:], in_=cs_d[:, t0 * C : (t0 + G) * C]
                        )
                        nc.vector.reduce_max(
                            S[:, t0 : t0 + G, :], csw[:], axis=AX.X
                        )
                    else:
                        csw = csp.tile([128, G, C], f16)
                        nc.sync.dma_start(
                            out=csw[:], in_=cs_d[:, t0 * C : (t0 + G) * C]
                        )
                        w1 = vp.tile([128, G, 512], f16)
                        nc.vector.tensor_tensor(
                            out=w1[:], in0=csw[:, :, 0:512],
                            in1=csw[:, :, 512:1024], op=OP.max,
                        )
                        w2 = vp.tile([128, G, 256], f16)
                        nc.vector.tensor_tensor(
                            out=w2[:], in0=w1[:, :, 0:256],
                            in1=w1[:, :, 256:512], op=OP.max,
                        )
                        w3 = vp.tile([128, G, 128], f16)
                        nc.vector.tensor_tensor(
                            out=w3[:], in0=w2[:, :, 0:128],
                            in1=w2[:, :, 128:256], op=OP.max,
                        )
                        w4 = vp.tile([128, G, 64], f16)
                        nc.vector.tensor_tensor(
                            out=w4[:], in0=w3[:, :, 0:64],
                            in1=w3[:, :, 64:128], op=OP.max,
                        )
                        w5 = vp.tile([128, G, 32], f16)
                        nc.vector.tensor_tensor(
                            out=w5[:], in0=w4[:, :, 0:32],
                            in1=w4[:, :, 32:64], op=OP.max,
                        )
                        w6 = vp.tile([128, G, 16], f16)
                        nc.vector.tensor_tensor(
                            out=w6[:], in0=w5[:, :, 0:16],
                            in1=w5[:, :, 16:32], op=OP.max,
                        )
                        nc.vector.tensor_tensor(
                            out=S[:, t0 : t0 + G, :], in0=w6[:, :, 0:8],
                            in1=w6[:, :, 8:16], op=OP.max,
                        )

                # masked row-max reconstruction
                nc.vector.reduce_max(m1[:], S[:], axis=AX.X)
                nc.vector.tensor_add(S2[:], S[:], p8[:])
                nc.vector.reduce_max(vd[:], S2[:], axis=AX.X)
                nc.vector.tensor_tensor(
                    out=eq[:], in0=v_sb, in1=m1[:], op=OP.is_equal
                )
                nc.vector.tensor_sub(dd[:], vd[:], m1[:])
                nc.vector.tensor_mul(dd[:], eq[:], dd[:])
                nc.vector.tensor_add(mm[:], m1[:], dd[:])
                nc.scalar.activation(
                    out=lm[:], in_=mm[:], func=AF.Ln,
                    bias=1.0, scale=1.0 / SCALE,
                )
                nc.vector.scalar_tensor_tensor(
                    out=scr[:], in0=w_sb[:], scalar=0.0, in1=lm[:],
                    op0=OP.bypass, op1=OP.mult, accum_out=out_sb[:, 0:1],
                )
            nc.scalar.dma_start(out=out_d[:], in_=out_sb[:])

    nc.compile()
    return nc


def make_in_maps(class_scores, xywh, z, r, nearest_gt_idx, gt_class_labels, gt_xywh):
    cs_f = np.ascontiguousarray(np.asarray(class_scores, dtype=np.float32))
    cs16 = ((cs_f - 1.0) * SCALE).astype(np.float16)
    xywh = np.ascontiguousarray(np.asarray(xywh, dtype=np.float32))
    z = np.ascontiguousarray(np.asarray(z, dtype=np.float32))
    r = np.ascontiguousarray(np.asarray(r, dtype=np.float32))
    idx = np.asarray(nearest_gt_idx).astype(np.int64)
    labels = np.asarray(gt_class_labels).astype(np.int64)[idx]           # [N]
    gx = np.asarray(gt_xywh, dtype=np.float32)[idx]                      # [N,4]
    v = cs16[np.arange(N), labels].astype(np.float32)                    # [N]
    kstar = (labels >> 7).astype(np.int64)                               # [N]
    p8_full = np.zeros((N, NB), dtype=np.float16)
    p8_full[np.arange(N), kstar] = PEN

    in_maps = []
    for c in range(NCORES):
        lo, hi = c * R, (c + 1) * R
        if hi <= N:
            cs_s = cs16[lo:hi]
            v_s, z_s, r_s = v[lo:hi], z[lo:hi], r[lo:hi]
            xywh_s, gx_s, p8_s = xywh[lo:hi], gx[lo:hi], p8_full[lo:hi]
        else:
            n_real = N - lo
            cs_s = np.zeros((R, C), dtype=np.float16)   # pad rows: cs'=0 -> log1p(0)=0
            cs_s[:n_real] = cs16[lo:]
            v_s = np.zeros(R, np.float32); v_s[:n_real] = v[lo:]
            z_s = np.zeros(R, np.float32); z_s[:n_real] = z[lo:]
            r_s = np.zeros(R, np.float32); r_s[:n_real] = r[lo:]
            xywh_s = np.zeros((R, 4), np.float32); xywh_s[:n_real] = xywh[lo:]
            gx_s = np.zeros((R, 4), np.float32); gx_s[:n_real] = gx[lo:]
            p8_s = np.zeros((R, NB), np.float16)
            p8_s[:, 0] = PEN
            p8_s[:n_real] = p8_full[lo:]
        pf = np.empty((128, PF_COLS), dtype=np.float32)
        pf[:, PF_V : PF_V + T] = v_s.reshape(128, T)
        pf[:, PF_Z : PF_Z + T] = z_s.reshape(128, T)
        pf[:, PF_R : PF_R + T] = r_s.reshape(128, T)
        pf[:, PF_XYWH : PF_XYWH + 4 * T] = xywh_s.reshape(128, 4 * T)
        pf[:, PF_G : PF_G + 4 * T] = gx_s.reshape(128, 4 * T)
        in_maps.append({
            "cs": np.ascontiguousarray(cs_s.reshape(128, T * C)),
            "pf": pf,
            "p8": np.ascontiguousarray(p8_s.reshape(128, T * NB)),
        })
    return in_maps


def combine_outputs(outs):
    """outs: list of [128, 2] per-core partials -> final [1] float32."""
    partA = float(sum(o[:, 0].astype(np.float64).sum() for o in outs))
    partB = float(sum(o[:, 1].astype(np.float64).sum() for o in outs))
    with np.errstate(over="ignore", under="ignore"):
        tps = np.exp(-partB)
    val = -partA + tps
    return np.array([val], dtype=np.float32)


_NC_CACHE = None


def get_nc():
    global _NC_CACHE
    if _NC_CACHE is None:
        _NC_CACHE = build_nc()
    return _NC_CACHE


def kernel(**inputs) -> np.ndarray:
    nc = get_nc()
    in_maps = make_in_maps(**inputs)
    res = run_bass_kernel_spmd(nc, in_maps, core_ids=list(range(NCORES)))
    return combine_outputs([res.results[c]["out"] for c in range(NCORES)])


# revision 13
# speedup vs baseline: 2.7842x; 1.5026x over previous
"""BPLoss Trainium2 kernel: 8-core SPMD over the detection (N) axis.

v4 design: fp16 streaming + maskless block-max + host-gathered label value.

Per core (shard of R=12544 rows; partition p owns rows p*98..p*98+97):
  - class_scores uploaded as fp16 (half the HBM traffic of v3), shifted and
    scaled on host to (cs - 1) * 1024 so the row max (~1 - 1e-3 for uniform
    scores) lands near magnitude ~1 with full fp16 relative precision
    instead of near 1.0 where fp16 spacing (4.9e-4) would swamp log(max).
    The device undoes it inside the Ln: log(masked/1024 + 1).
  - NO mask pass over the [N, 1024] matrix.  Instead, per 128-row tile the
    DVE computes 8 per-row block maxes S[k] = max(cs[:, 128k:128k+128]) in a
    single strided reduce_max per 7-tile group (fp16 2x mode).
  - Masked row-max reconstruction (exact unless the 2nd-largest element
    shares its 128-col block with the label AND the label is the argmax,
    ~25 rows in 100k, error ~1e-3 each):
        M1 = max_k S[k]
        Vd = max_k (S[k] + P8[k])   with P8 = -2 at the label's block
        masked = (v == M1) ? Vd : M1
    where v = cs_fp16[row, label] is gathered on host (O(N)) and P8 is a
    host-built [T, 8] fp16 penalty table.
  - epilogue: Ln on ScalarE, fused multiply-accumulate dot products for
    sum((z+r)*log_masked) and sum(z*||xywh - gt_xywh[idx]||^2).
Host: gathers the tiny gt tables per row, shards, pads core 7, sums the
8x[128,2] partials, combines -A + exp(-B).
"""
import numpy as np
import concourse.bass as bass
import concourse.tile as tile
from concourse import bacc, mybir
from concourse.bass_utils import run_bass_kernel_spmd

N, C, M = 100000, 1024, 128
NCORES = 8
T = 98              # 128-row tiles per core
R = T * 128         # 12544 rows per core
G = 7               # tiles per DMA group
NG = T // G         # 14 groups
NB = 8              # per-row column blocks (classes) of C//NB = 128 cols
CS_BUFS = 4
BULK = "tree"       # "reduce": strided reduce_max / "tree": tensor_tensor max
SCALE = 1024.0      # host uploads (cs - 1) * SCALE
PEN = -4096.0       # block penalty; dominates any real shifted score (>= -1024)

f16 = mybir.dt.float16
f32 = mybir.dt.float32
OP = mybir.AluOpType
AF = mybir.ActivationFunctionType
AX = mybir.AxisListType

# packed f32 per-row tables: [v | z | r | xywh | g]
PF_V = 0
PF_Z = T
PF_R = 2 * T
PF_XYWH = 3 * T
PF_G = 7 * T
PF_COLS = 11 * T


def build_nc(reps=1):
    nc = bacc.Bacc("TRN2", target_bir_lowering=False, debug=False,
                   num_devices=NCORES)
    cs_d = nc.dram_tensor("cs", [128, T * C], f16, kind="ExternalInput").ap()
    pf_d = nc.dram_tensor("pf", [128, PF_COLS], f32, kind="ExternalInput").ap()
    p8_d = nc.dram_tensor("p8", [128, T * NB], f16, kind="ExternalInput").ap()
    out_d = nc.dram_tensor("out", [128, 2], f32, kind="ExternalOutput").ap()

    with tile.TileContext(nc) as tc:
        with (
            tc.tile_pool(name="const", bufs=1) as constp,
            tc.tile_pool(name="csp", bufs=CS_BUFS) as csp,
            tc.tile_pool(name="vp", bufs=2) as vp,
        ):
            pf = constp.tile([128, PF_COLS], f32)
            nc.scalar.dma_start(out=pf[:], in_=pf_d[:])
            p8 = constp.tile([128, T, NB], f16)
            nc.scalar.dma_start(out=p8[:], in_=p8_d[:])
            v_sb = pf[:, PF_V : PF_V + T]
            z_sb = pf[:, PF_Z : PF_Z + T]
            r_sb = pf[:, PF_R : PF_R + T]
            xywh_sb = pf[:, PF_XYWH : PF_XYWH + 4 * T].rearrange(
                "p (t c) -> p t c", c=4
            )
            g_sb = pf[:, PF_G : PF_G + 4 * T].rearrange("p (t c) -> p t c", c=4)

            S = constp.tile([128, T, NB], f16)
            S2 = constp.tile([128, T, NB], f16)
            B3 = constp.tile([128, T, 128], f16)
            w4t = constp.tile([128, T, 64], f16)
            w5t = constp.tile([128, T, 32], f16)
            w6t = constp.tile([128, T, 16], f16)
            m1 = constp.tile([128, T], f32)
            vd = constp.tile([128, T], f32)
            eq = constp.tile([128, T], f32)
            dd = constp.tile([128, T], f32)
            mm = constp.tile([128, T], f32)
            lm = constp.tile([128, T], f32)
            w_sb = constp.tile([128, T], f32)
            scr = constp.tile([128, T], f32)
            scr2 = constp.tile([128, T], f32)
            diff = constp.tile([128, T, 4], f32)
            dsum = constp.tile([128, T], f32)
            out_sb = constp.tile([128, 2], f32)
            warm = constp.tile([128, 1], f32)
            warm2 = constp.tile([128, 1], f32)

            # preload the Ln activation table while DMAs stream
            nc.vector.memset(warm[:], 1.0)
            nc.scalar.activation(out=warm2[:], in_=warm[:], func=AF.Ln)

            for rep in range(reps):
                # shape-loss term first: independent of class_scores
                nc.vector.tensor_add(w_sb[:], z_sb, r_sb)
                nc.vector.tensor_sub(diff[:], xywh_sb, g_sb)
                nc.vector.tensor_mul(diff[:], diff[:], diff[:])
                nc.vector.reduce_sum(dsum[:], diff[:], axis=AX.X)
                nc.vector.scalar_tensor_tensor(
                    out=scr2[:], in0=z_sb, scalar=0.0, in1=dsum[:],
                    op0=OP.bypass, op1=OP.mult, accum_out=out_sb[:, 1:2],
                )

                def tail(ta, tb):
                    """tree levels 4-7 + per-row S for tiles [ta, tb)."""
                    nc.vector.tensor_tensor(
                        out=w4t[:, ta:tb, :], in0=B3[:, ta:tb, 0:64],
                        in1=B3[:, ta:tb, 64:128], op=OP.max,
                    )
                    nc.vector.tensor_tensor(
                        out=w5t[:, ta:tb, :], in0=w4t[:, ta:tb, 0:32],
                        in1=w4t[:, ta:tb, 32:64], op=OP.max,
                    )
                    nc.vector.tensor_tensor(
                        out=w6t[:, ta:tb, :], in0=w5t[:, ta:tb, 0:16],
                        in1=w5t[:, ta:tb, 16:32], op=OP.max,
                    )
                    nc.vector.tensor_tensor(
                        out=S[:, ta:tb, :], in0=w6t[:, ta:tb, 0:8],
                        in1=w6t[:, ta:tb, 8:16], op=OP.max,
                    )

                for g in range(NG):
                    t0 = g * G
                    if BULK == "reduce":
                        csw = csp.tile([128, G, NB, C // NB], f16)
                        nc.sync.dma_start(
                            out=csw[:], in_=cs_d[:, t0 * C : (t0 + G) * C]
                        )
                        nc.vector.reduce_max(
                            S[:, t0 : t0 + G, :], csw[:], axis=AX.X
                        )
                    else:
                        csw = csp.tile([128, G, C], f16)
                        nc.sync.dma_start(
                            out=csw[:], in_=cs_d[:, t0 * C : (t0 + G) * C]
                        )
                        w1 = vp.tile([128, G, 512], f16)
                        nc.vector.tensor_tensor(
                            out=w1[:], in0=csw[:, :, 0:512],
                            in1=csw[:, :, 512:1024], op=OP.max,
                        )
                        w2 = vp.tile([128, G, 256], f16)
                        nc.vector.tensor_tensor(
                            out=w2[:], in0=w1[:, :, 0:256],
                            in1=w1[:, :, 256:512], op=OP.max,
                        )
                        nc.vector.tensor_tensor(
                            out=B3[:, t0 : t0 + G, :], in0=w2[:, :, 0:128],
                            in1=w2[:, :, 128:256], op=OP.max,
                        )
                        if g == NG // 2 - 1:
                            tail(0, (NG // 2) * G)
                if BULK == "tree":
                    tail((NG // 2) * G, T)

                # masked row-max reconstruction
                nc.vector.reduce_max(m1[:], S[:], axis=AX.X)
                nc.vector.tensor_add(S2[:], S[:], p8[:])
                nc.vector.reduce_max(vd[:], S2[:], axis=AX.X)
                # masked = max(Vd, M1 - BIG*(v==M1)); Vd <= M1 always
                nc.vector.tensor_tensor(
                    out=eq[:], in0=v_sb, in1=m1[:], op=OP.is_equal
                )
                nc.vector.scalar_tensor_tensor(
                    out=dd[:], in0=eq[:], scalar=-1.0e9, in1=m1[:],
                    op0=OP.mult, op1=OP.add,
                )
                nc.vector.tensor_tensor(
                    out=mm[:], in0=dd[:], in1=vd[:], op=OP.max
                )
                nc.scalar.activation(
                    out=lm[:], in_=mm[:], func=AF.Ln,
                    bias=1.0, scale=1.0 / SCALE,
                )
                nc.vector.scalar_tensor_tensor(
                    out=scr[:], in0=w_sb[:], scalar=0.0, in1=lm[:],
                    op0=OP.bypass, op1=OP.mult, accum_out=out_sb[:, 0:1],
                )
            nc.scalar.dma_start(out=out_d[:], in_=out_sb[:])

    nc.compile()
    return nc


def make_in_maps(class_scores, xywh, z, r, nearest_gt_idx, gt_class_labels, gt_xywh):
    cs_f = np.ascontiguousarray(np.asarray(class_scores, dtype=np.float32))
    cs16 = ((cs_f - 1.0) * SCALE).astype(np.float16)
    xywh = np.ascontiguousarray(np.asarray(xywh, dtype=np.float32))
    z = np.ascontiguousarray(np.asarray(z, dtype=np.float32))
    r = np.ascontiguousarray(np.asarray(r, dtype=np.float32))
    idx = np.asarray(nearest_gt_idx).astype(np.int64)
    labels = np.asarray(gt_class_labels).astype(np.int64)[idx]           # [N]
    gx = np.asarray(gt_xywh, dtype=np.float32)[idx]                      # [N,4]
    v = cs16[np.arange(N), labels].astype(np.float32)                    # [N]
    kstar = (labels >> 7).astype(np.int64)                               # [N]
    p8_full = np.zeros((N, NB), dtype=np.float16)
    p8_full[np.arange(N), kstar] = PEN

    in_maps = []
    for c in range(NCORES):
        lo, hi = c * R, (c + 1) * R
        if hi <= N:
            cs_s = cs16[lo:hi]
            v_s, z_s, r_s = v[lo:hi], z[lo:hi], r[lo:hi]
            xywh_s, gx_s, p8_s = xywh[lo:hi], gx[lo:hi], p8_full[lo:hi]
        else:
            n_real = N - lo
            cs_s = np.zeros((R, C), dtype=np.float16)   # pad rows: cs'=0 -> log1p(0)=0
            cs_s[:n_real] = cs16[lo:]
            v_s = np.zeros(R, np.float32); v_s[:n_real] = v[lo:]
            z_s = np.zeros(R, np.float32); z_s[:n_real] = z[lo:]
            r_s = np.zeros(R, np.float32); r_s[:n_real] = r[lo:]
            xywh_s = np.zeros((R, 4), np.float32); xywh_s[:n_real] = xywh[lo:]
            gx_s = np.zeros((R, 4), np.float32); gx_s[:n_real] = gx[lo:]
            p8_s = np.zeros((R, NB), np.float16)
            p8_s[:, 0] = PEN
            p8_s[:n_real] = p8_full[lo:]
        pf = np.empty((128, PF_COLS), dtype=np.float32)
        pf[:, PF_V : PF_V + T] = v_s.reshape(128, T)
        pf[:, PF_Z : PF_Z + T] = z_s.reshape(128, T)
        pf[:, PF_R : PF_R + T] = r_s.reshape(128, T)
        pf[:, PF_XYWH : PF_XYWH + 4 * T] = xywh_s.reshape(128, 4 * T)
        pf[:, PF_G : PF_G + 4 * T] = gx_s.reshape(128, 4 * T)
        in_maps.append({
            "cs": np.ascontiguousarray(cs_s.reshape(128, T * C)),
            "pf": pf,
            "p8": np.ascontiguousarray(p8_s.reshape(128, T * NB)),
        })
    return in_maps


def combine_outputs(outs):
    """outs: list of [128, 2] per-core partials -> final [1] float32."""
    partA = float(sum(o[:, 0].astype(np.float64).sum() for o in outs))
    partB = float(sum(o[:, 1].astype(np.float64).sum() for o in outs))
    with np.errstate(over="ignore", under="ignore"):
        tps = np.exp(-partB)
    val = -partA + tps
    return np.array([val], dtype=np.float32)


_NC_CACHE = None


def get_nc():
    global _NC_CACHE
    if _NC_CACHE is None:
        _NC_CACHE = build_nc()
    return _NC_CACHE


def kernel(**inputs) -> np.ndarray:
    nc = get_nc()
    in_maps = make_in_maps(**inputs)
    res = run_bass_kernel_spmd(nc, in_maps, core_ids=list(range(NCORES)))
    return combine_outputs([res.results[c]["out"] for c in range(NCORES)])


# revision 21
# speedup vs baseline: 2.8034x; 1.0069x over previous
"""BPLoss Trainium2 kernel: 8-core SPMD over the detection (N) axis.

v4 design: fp16 streaming + maskless block-max + host-gathered label value.

Per core (shard of R=12544 rows; partition p owns rows p*98..p*98+97):
  - class_scores uploaded as fp16 (half the HBM traffic of v3), shifted and
    scaled on host to (cs - 1) * 1024 so the row max (~1 - 1e-3 for uniform
    scores) lands near magnitude ~1 with full fp16 relative precision
    instead of near 1.0 where fp16 spacing (4.9e-4) would swamp log(max).
    The device undoes it inside the Ln: log(masked/1024 + 1).
  - NO mask pass over the [N, 1024] matrix.  Instead, per 128-row tile the
    DVE computes 8 per-row block maxes S[k] = max(cs[:, 128k:128k+128]) in a
    single strided reduce_max per 7-tile group (fp16 2x mode).
  - Masked row-max reconstruction (exact unless the 2nd-largest element
    shares its 128-col block with the label AND the label is the argmax,
    ~25 rows in 100k, error ~1e-3 each):
        M1 = max_k S[k]
        Vd = max_k (S[k] + P8[k])   with P8 = -2 at the label's block
        masked = (v == M1) ? Vd : M1
    where v = cs_fp16[row, label] is gathered on host (O(N)) and P8 is a
    host-built [T, 8] fp16 penalty table.
  - epilogue: Ln on ScalarE, fused multiply-accumulate dot products for
    sum((z+r)*log_masked) and sum(z*||xywh - gt_xywh[idx]||^2).
Host: gathers the tiny gt tables per row, shards, pads core 7, sums the
8x[128,2] partials, combines -A + exp(-B).
"""
import numpy as np
import concourse.bass as bass
import concourse.tile as tile
from concourse import bacc, mybir
from concourse.bass_utils import run_bass_kernel_spmd

N, C, M = 100000, 1024, 128
NCORES = 8
T = 98              # 128-row tiles per core
R = T * 128         # 12544 rows per core
G = 7               # tiles per DMA group
NG = T // G         # 14 groups
NB = 8              # per-row column blocks (classes) of C//NB = 128 cols
CS_BUFS = 6
BULK = "tree"       # "reduce": strided reduce_max / "tree": tensor_tensor max
SCALE = 1024.0      # host uploads (cs - 1) * SCALE
PEN = -4096.0       # block penalty; dominates any real shifted score (>= -1024)

f16 = mybir.dt.float16
f32 = mybir.dt.float32
OP = mybir.AluOpType
AF = mybir.ActivationFunctionType
AX = mybir.AxisListType

# packed f32 per-row tables: [v | z | r | xywh | g]
PF_V = 0
PF_Z = T
PF_R = 2 * T
PF_XYWH = 3 * T
PF_G = 7 * T
PF_COLS = 11 * T


def build_nc(reps=1):
    nc = bacc.Bacc("TRN2", target_bir_lowering=False, debug=False,
                   num_devices=NCORES)
    cs_d = nc.dram_tensor("cs", [128, T * C], f16, kind="ExternalInput").ap()
    pf_d = nc.dram_tensor("pf", [128, PF_COLS], f32, kind="ExternalInput").ap()
    p8_d = nc.dram_tensor("p8", [128, T * NB], f16, kind="ExternalInput").ap()
    out_d = nc.dram_tensor("out", [128, 3], f32, kind="ExternalOutput").ap()

    with tile.TileContext(nc) as tc:
        with (
            tc.tile_pool(name="const", bufs=1) as constp,
            tc.tile_pool(name="csp", bufs=CS_BUFS) as csp,
            tc.tile_pool(name="vp", bufs=2) as vp,
        ):
            pf = constp.tile([128, PF_COLS], f32)
            nc.scalar.dma_start(out=pf[:], in_=pf_d[:])
            p8 = constp.tile([128, T, NB], f16)
            nc.scalar.dma_start(out=p8[:], in_=p8_d[:])
            v_sb = pf[:, PF_V : PF_V + T]
            z_sb = pf[:, PF_Z : PF_Z + T]
            r_sb = pf[:, PF_R : PF_R + T]
            xywh_sb = pf[:, PF_XYWH : PF_XYWH + 4 * T].rearrange(
                "p (t c) -> p t c", c=4
            )
            g_sb = pf[:, PF_G : PF_G + 4 * T].rearrange("p (t c) -> p t c", c=4)

            S = constp.tile([128, T, NB], f16)
            S2 = constp.tile([128, T, NB], f16)
            B3 = constp.tile([128, T, 128], f16)
            w4t = constp.tile([128, T, 64], f16)
            w5t = constp.tile([128, T, 32], f16)
            w6t = constp.tile([128, T, 16], f16)
            m1 = constp.tile([128, T], f32)
            vd = constp.tile([128, T], f32)
            eq = constp.tile([128, T], f32)
            dd = constp.tile([128, T], f32)
            mm = constp.tile([128, T], f32)
            lm = constp.tile([128, T], f32)
            w_sb = constp.tile([128, T], f32)
            scr = constp.tile([128, T], f32)
            scr2 = constp.tile([128, T], f32)
            diff = constp.tile([128, T, 4], f32)
            dsum = constp.tile([128, T], f32)
            out_sb = constp.tile([128, 3], f32)
            warm = constp.tile([128, 1], f32)
            warm2 = constp.tile([128, 1], f32)

            # preload the Ln activation table while DMAs stream
            nc.vector.memset(warm[:], 1.0)
            nc.scalar.activation(out=warm2[:], in_=warm[:], func=AF.Ln)
            nc.vector.memset(out_sb[:], 0.0)

            for rep in range(reps):

                def shape_term():
                    """sum(z * ||xywh - gt||^2): independent of class_scores."""
                    nc.vector.tensor_add(w_sb[:], z_sb, r_sb)
                    nc.vector.tensor_sub(diff[:], xywh_sb, g_sb)
                    nc.vector.tensor_mul(diff[:], diff[:], diff[:])
                    nc.vector.reduce_sum(dsum[:], diff[:], axis=AX.X)
                    nc.vector.scalar_tensor_tensor(
                        out=scr2[:], in0=z_sb, scalar=0.0, in1=dsum[:],
                        op0=OP.bypass, op1=OP.mult, accum_out=out_sb[:, 2:3],
                    )

                def tail(ta, tb):
                    """tree levels 4-7 + per-row S for tiles [ta, tb)."""
                    nc.vector.tensor_tensor(
                        out=w4t[:, ta:tb, :], in0=B3[:, ta:tb, 0:64],
                        in1=B3[:, ta:tb, 64:128], op=OP.max,
                    )
                    nc.vector.tensor_tensor(
                        out=w5t[:, ta:tb, :], in0=w4t[:, ta:tb, 0:32],
                        in1=w4t[:, ta:tb, 32:64], op=OP.max,
                    )
                    nc.vector.tensor_tensor(
                        out=w6t[:, ta:tb, :], in0=w5t[:, ta:tb, 0:16],
                        in1=w5t[:, ta:tb, 16:32], op=OP.max,
                    )
                    nc.vector.tensor_tensor(
                        out=S[:, ta:tb, :], in0=w6t[:, ta:tb, 0:8],
                        in1=w6t[:, ta:tb, 8:16], op=OP.max,
                    )

                def epilogue(ta, tb, col):
                    """masked row-max + log dot for tiles [ta, tb)."""
                    nc.vector.reduce_max(
                        m1[:, ta:tb], S[:, ta:tb, :], axis=AX.X
                    )
                    nc.vector.tensor_add(
                        S2[:, ta:tb, :], S[:, ta:tb, :], p8[:, ta:tb, :]
                    )
                    nc.vector.reduce_max(
                        vd[:, ta:tb], S2[:, ta:tb, :], axis=AX.X
                    )
                    # masked = max(Vd, M1 - BIG*(v==M1)); Vd <= M1 always
                    nc.vector.tensor_tensor(
                        out=eq[:, ta:tb], in0=v_sb[:, ta:tb],
                        in1=m1[:, ta:tb], op=OP.is_equal,
                    )
                    nc.vector.scalar_tensor_tensor(
                        out=dd[:, ta:tb], in0=eq[:, ta:tb], scalar=-1.0e9,
                        in1=m1[:, ta:tb], op0=OP.mult, op1=OP.add,
                    )
                    nc.vector.tensor_tensor(
                        out=mm[:, ta:tb], in0=dd[:, ta:tb],
                        in1=vd[:, ta:tb], op=OP.max,
                    )
                    nc.scalar.activation(
                        out=lm[:, ta:tb], in_=mm[:, ta:tb], func=AF.Ln,
                        bias=1.0, scale=1.0 / SCALE,
                    )
                    nc.vector.scalar_tensor_tensor(
                        out=scr[:, ta:tb], in0=w_sb[:, ta:tb], scalar=0.0,
                        in1=lm[:, ta:tb], op0=OP.bypass, op1=OP.mult,
                        accum_out=out_sb[:, col : col + 1],
                    )

                # tail chunks keyed on group completion: after group g,
                # tiles [0, (g+1)*G) are in B3.
                tails = {3: (0, 28), 6: (28, 49), 10: (49, 77), 13: (77, 98)}

                for g in range(NG):
                    t0 = g * G
                    if BULK == "reduce":
                        csw = csp.tile([128, G, NB, C // NB], f16)
                        nc.sync.dma_start(
                            out=csw[:], in_=cs_d[:, t0 * C : (t0 + G) * C]
                        )
                        nc.vector.reduce_max(
                            S[:, t0 : t0 + G, :], csw[:], axis=AX.X
                        )
                    else:
                        csw = csp.tile([128, G, C], f16)
                        nc.sync.dma_start(
                            out=csw[:], in_=cs_d[:, t0 * C : (t0 + G) * C]
                        )
                        w1 = vp.tile([128, G, 512], f16)
                        nc.vector.tensor_tensor(
                            out=w1[:], in0=csw[:, :, 0:512],
                            in1=csw[:, :, 512:1024], op=OP.max,
                        )
                        w2 = vp.tile([128, G, 256], f16)
                        nc.vector.tensor_tensor(
                            out=w2[:], in0=w1[:, :, 0:256],
                            in1=w1[:, :, 256:512], op=OP.max,
                        )
                        nc.vector.tensor_tensor(
                            out=B3[:, t0 : t0 + G, :], in0=w2[:, :, 0:128],
                            in1=w2[:, :, 128:256], op=OP.max,
                        )
                        if g in tails:
                            tail(*tails[g])
                        if g == 4:
                            shape_term()
                        if g == 7:
                            epilogue(0, 49, 0)
                if BULK == "tree":
                    epilogue(49, T, 1)
                else:
                    shape_term()
                    epilogue(0, T, 0)
            nc.scalar.dma_start(out=out_d[:], in_=out_sb[:])

    nc.compile()
    return nc


def make_in_maps(class_scores, xywh, z, r, nearest_gt_idx, gt_class_labels, gt_xywh):
    cs_f = np.ascontiguousarray(np.asarray(class_scores, dtype=np.float32))
    cs16 = ((cs_f - 1.0) * SCALE).astype(np.float16)
    xywh = np.ascontiguousarray(np.asarray(xywh, dtype=np.float32))
    z = np.ascontiguousarray(np.asarray(z, dtype=np.float32))
    r = np.ascontiguousarray(np.asarray(r, dtype=np.float32))
    idx = np.asarray(nearest_gt_idx).astype(np.int64)
    labels = np.asarray(gt_class_labels).astype(np.int64)[idx]           # [N]
    gx = np.asarray(gt_xywh, dtype=np.float32)[idx]                      # [N,4]
    v = cs16[np.arange(N), labels].astype(np.float32)                    # [N]
    # class k of S = columns congruent to k mod 8 when BULK == "tree"
    # (pairwise fold halves the column index range each level), contiguous
    # 128-col blocks when BULK == "reduce".
    if BULK == "tree":
        kstar = (labels & 7).astype(np.int64)                            # [N]
    else:
        kstar = (labels >> 7).astype(np.int64)                           # [N]
    p8_full = np.zeros((N, NB), dtype=np.float16)
    p8_full[np.arange(N), kstar] = PEN

    in_maps = []
    for c in range(NCORES):
        lo, hi = c * R, (c + 1) * R
        if hi <= N:
            cs_s = cs16[lo:hi]
            v_s, z_s, r_s = v[lo:hi], z[lo:hi], r[lo:hi]
            xywh_s, gx_s, p8_s = xywh[lo:hi], gx[lo:hi], p8_full[lo:hi]
        else:
            n_real = N - lo
            cs_s = np.zeros((R, C), dtype=np.float16)   # pad rows: cs'=0 -> log1p(0)=0
            cs_s[:n_real] = cs16[lo:]
            v_s = np.zeros(R, np.float32); v_s[:n_real] = v[lo:]
            z_s = np.zeros(R, np.float32); z_s[:n_real] = z[lo:]
            r_s = np.zeros(R, np.float32); r_s[:n_real] = r[lo:]
            xywh_s = np.zeros((R, 4), np.float32); xywh_s[:n_real] = xywh[lo:]
            gx_s = np.zeros((R, 4), np.float32); gx_s[:n_real] = gx[lo:]
            p8_s = np.zeros((R, NB), np.float16)
            p8_s[:, 0] = PEN
            p8_s[:n_real] = p8_full[lo:]
        pf = np.empty((128, PF_COLS), dtype=np.float32)
        pf[:, PF_V : PF_V + T] = v_s.reshape(128, T)
        pf[:, PF_Z : PF_Z + T] = z_s.reshape(128, T)
        pf[:, PF_R : PF_R + T] = r_s.reshape(128, T)
        pf[:, PF_XYWH : PF_XYWH + 4 * T] = xywh_s.reshape(128, 4 * T)
        pf[:, PF_G : PF_G + 4 * T] = gx_s.reshape(128, 4 * T)
        in_maps.append({
            "cs": np.ascontiguousarray(cs_s.reshape(128, T * C)),
            "pf": pf,
            "p8": np.ascontiguousarray(p8_s.reshape(128, T * NB)),
        })
    return in_maps


def combine_outputs(outs):
    """outs: list of [128, 3] per-core partials -> final [1] float32."""
    partA = float(sum((o[:, 0] + o[:, 1]).astype(np.float64).sum() for o in outs))
    partB = float(sum(o[:, 2].astype(np.float64).sum() for o in outs))
    with np.errstate(over="ignore", under="ignore"):
        tps = np.exp(-partB)
    val = -partA + tps
    return np.array([val], dtype=np.float32)


_NC_CACHE = None


def get_nc():
    global _NC_CACHE
    if _NC_CACHE is None:
        _NC_CACHE = build_nc()
    return _NC_CACHE


def kernel(**inputs) -> np.ndarray:
    nc = get_nc()
    in_maps = make_in_maps(**inputs)
    res = run_bass_kernel_spmd(nc, in_maps, core_ids=list(range(NCORES)))
    return combine_outputs([res.results[c]["out"] for c in range(NCORES)])
